# revision 1
# baseline (speedup 1.0000x reference)
"""Gabor-atom synthesis, multirate variant: time-sharded across 8 cores.

Each core renders ALL 2048 atoms over its 6000-sample slice. Atoms are
sorted by carrier frequency into 16 blocks of 128; low-frequency blocks are
synthesized on a decimated grid (rate r in {2,4,8}, chosen per block from
the actual omegas at build time: omega_max <= 0.35*fs/r) and upsampled with
a Kaiser-windowed polyphase FIR folded into PE matmuls. The phase pipeline
per block is the same DDS scheme as the full-rate kernel (fix 2^23 f32 FMA
-> i32, and/or mantissa bits, Sin on bitcast), just on the coarse grid.

Stage 1 (per fine 120-chunk): PE reduces sin over each block with the
(-amp*env, -amp*denv) moving pair. Full-rate blocks accumulate straight
into the output PSUM; each decimated rate class reduces into its own
[120/r + 2H, 2] coarse window (windows overlap by the FIR halo H=6 so no
cross-chunk stitching is needed). Stage 2 (end): coarse windows combine
(env interp) to bf16, then one FIR matmul per (class, chunk) accumulates
the upsampled contribution into a separate PSUM; the final DVE combine sums
r0 + (i/120)*rd + fir and DMAs out.
"""
import numpy as np
import ml_dtypes
from contextlib import ExitStack

import concourse.bacc as bacc
import concourse.tile as tile
from concourse import mybir
from concourse.bass_utils import run_bass_kernel_spmd

FS = 24000.0
T = 48000
N_ATOMS = 2048
N_CORES = 8
NYQUIST = FS / 2.0
SIGMA_OFFSET = 1e-3

P = 128
NB = 16                      # atom blocks (all on every core)
TC = T // N_CORES            # 6000 fine samples per core
RCH = 120                    # fine reduce chunk
N_COLS = TC // RCH           # 50 output columns per core
TW = 600                     # fine tile width
N_TILES = TC // TW           # 10
CPT = TW // RCH              # fine chunks per tile (5)
FIX = 2 ** 23
H = 8                        # FIR halo (coarse samples per side)
FIR_CUT = 0.85               # FIR cutoff (fraction of coarse Nyquist)
RATE_MARG = 0.35             # block rate rule: omega_max <= RATE_MARG*fs/r

f32 = mybir.dt.float32
f16 = mybir.dt.float16
i32 = mybir.dt.int32
bf16 = mybir.dt.bfloat16
bft = ml_dtypes.bfloat16

_cache = {}


def _fir(r):
    """Interp kernel h (gain r), length 2*H*r+1, via Kaiser-windowed sinc."""
    n = np.arange(-H * r, H * r + 1, dtype=np.float64)
    x = n / r
    c = FIR_CUT / r * (1.0)  # cutoff as fraction of coarse Nyquist -> sinc arg
    hh = FIR_CUT * np.sinc(FIR_CUT * x)
    beta = 8.0
    w = np.i0(beta * np.sqrt(np.maximum(0.0, 1 - (x / H) ** 2))) / np.i0(beta)
    return hh * w


def _fir_matrix(r):
    """FIRM[k, i]: coarse window sample k -> fine chunk sample i."""
    h = _fir(r)
    wr = RCH // r + 2 * H
    M = np.zeros((wr, RCH), np.float64)
    for k in range(wr):
        for i in range(RCH):
            idx = i + 2 * H * r - k * r   # center Hr + (i/r + H - k)*r
            if 0 <= idx < h.size:
                M[k, i] = h[idx]
    return M


def _build_program(rates):
    """rates: tuple of NB ints (1/2/4/8), block g -> decimation rate."""
    nc = bacc.Bacc("TRN2", target_bir_lowering=False, debug=False)

    classes = sorted({r for r in rates if r > 1})
    wg = [TW // r + (0 if r == 1 else 2 * H) for r in rates]  # tile cols/blk
    off = np.concatenate([[0], np.cumsum(wg)]).astype(int)
    WT = int(off[-1])                       # total tile width
    wc = {r: RCH // r + 2 * H for r in classes}   # coarse window per chunk

    d_bi = nc.dram_tensor("bi", [P, NB * N_TILES], f32, kind="ExternalInput").ap()
    d_ci = nc.dram_tensor("ci", [P, NB * N_TILES], f32, kind="ExternalInput").ap()
    d_envp = nc.dram_tensor("envp", [P, NB * 2 * N_COLS], bf16,
                            kind="ExternalInput").ap()
    d_firm = {r: nc.dram_tensor(f"firm{r}", [wc[r], RCH], f16,
                                kind="ExternalInput").ap() for r in classes}
    d_firm2 = {r: nc.dram_tensor(f"firm2{r}", [wc[r], RCH], f16,
                                 kind="ExternalInput").ap() for r in classes}
    d_out = nc.dram_tensor("wave", [RCH, N_COLS], f32, kind="ExternalOutput").ap()

    with tile.TileContext(nc) as tc, ExitStack() as ctx:
        consts = ctx.enter_context(tc.tile_pool(name="consts", bufs=1))
        kpool = ctx.enter_context(tc.tile_pool(name="kp", bufs=4))
        mpool = ctx.enter_context(tc.tile_pool(name="mp", bufs=2))
        spool = ctx.enter_context(tc.tile_pool(name="sp", bufs=2))
        cwpool = ctx.enter_context(tc.tile_pool(name="cw", bufs=1))
        opool = ctx.enter_context(tc.tile_pool(name="op", bufs=1))
        rpool = ctx.enter_context(tc.tile_pool(name="rp", bufs=1, space="PSUM"))

        # ramps (one per distinct width), centered
        widths = sorted({wg[g] for g in range(NB)})
        widths = [widths[0]] + widths[-1:] + widths[1:-1]
        t_ramp = {}
        for w in widths:
            t_r = consts.tile([P, w], i32, name=f"ramp{w}")
            nc.gpsimd.iota(t_r[:], [[1, w]], base=-(w // 2), channel_multiplier=0)
            t_ramp[w] = t_r
        t_pi = consts.tile([P, 1], i32)
        nc.gpsimd.iota(t_pi[:], [[0, 1]], base=0, channel_multiplier=1)

        t_bias = consts.tile([P, 1], f32)
        nc.vector.memset(t_bias[:], float(-3.0 * np.pi))
        t_warm = consts.tile([P, 1], bf16)
        nc.scalar.activation(t_warm[:], t_bias[:],
                             mybir.ActivationFunctionType.Sin, scale=1.0)

        t_bi = consts.tile([P, NB * N_TILES], f32)
        nc.sync.dma_start(t_bi[:], d_bi[:])
        t_ci = consts.tile([P, NB * N_TILES], f32)
        nc.sync.dma_start(t_ci[:], d_ci[:])
        t_envp = consts.tile([P, NB * 2 * N_COLS], bf16)
        nc.sync.dma_start(t_envp[:], d_envp[:])
        t_firm = {}
        t_firm2 = {}
        for r in classes:
            t_f = consts.tile([wc[r], RCH], f16, name=f"tf{r}")
            nc.sync.dma_start(t_f[:], d_firm[r][:])
            t_firm[r] = t_f
            t_f2 = consts.tile([wc[r], RCH], f16, name=f"tf2{r}")
            nc.sync.dma_start(t_f2[:], d_firm2[r][:])
            t_firm2[r] = t_f2

        # fine env-interp weight i/120 and coarse weights (k-H)/(120/r)
        t_w = consts.tile([P, 1], f32)
        nc.vector.tensor_scalar(t_w[:], t_pi[:], float(1.0 / RCH), None,
                                mybir.AluOpType.mult)

        p_r = rpool.tile([P, 2 * N_COLS], f32)
        p_c = {r: rpool.tile([wc[r], 2 * N_COLS], f32, name=f"pc{r}")
               for r in classes}
        p_fir = rpool.tile([P, N_COLS], f32, name="p_fir") if classes else None

        def fma_item(t_k, g, t, eng):
            col = g * N_TILES + t
            w = wg[g]
            eng.tensor_scalar(
                t_k[:, int(off[g]):int(off[g]) + w],
                t_ramp[w][:], t_bi[:, col:col + 1], t_ci[:, col:col + 1],
                mybir.AluOpType.mult, mybir.AluOpType.add)

        def andor(t_m, t_k, lo, hi):
            nc.vector.tensor_scalar(
                t_m[:, lo:hi], t_k[:, lo:hi], 0x7FFFFF, 0x3F800000,
                mybir.AluOpType.bitwise_and, mybir.AluOpType.bitwise_or)

        def sin(t_s, t_m, lo, hi):
            nc.scalar.activation(
                t_s[:, lo:hi], t_m[:, lo:hi].bitcast(f32),
                mybir.ActivationFunctionType.Sin,
                scale=float(2.0 * np.pi), bias=t_bias[:, 0:1])

        def reduce_tile(t_s, t):
            for cl in range(CPT):
                C = t * CPT + cl
                r1 = [g for g in range(NB) if rates[g] == 1]
                for j, g in enumerate(r1):
                    nc.tensor.matmul(
                        p_r[:RCH, 2 * C:2 * C + 2],
                        t_s[:, int(off[g]) + cl * RCH:
                            int(off[g]) + (cl + 1) * RCH],
                        t_envp[:, g * 2 * N_COLS + 2 * C:
                               g * 2 * N_COLS + 2 * C + 2],
                        start=(j == 0), stop=(j == len(r1) - 1))
                for r in classes:
                    gs = [g for g in range(NB) if rates[g] == r]
                    for j, g in enumerate(gs):
                        nc.tensor.matmul(
                            p_c[r][:, 2 * C:2 * C + 2],
                            t_s[:, int(off[g]) + cl * (RCH // r):
                                int(off[g]) + cl * (RCH // r) + wc[r]],
                            t_envp[:, g * 2 * N_COLS + 2 * C:
                                   g * 2 * N_COLS + 2 * C + 2],
                            start=(j == 0), stop=(j == len(gs) - 1))

        # --- tile 0 in four 4-block subgroups (fast ACT start); the first
        # subgroup (lowest blocks) is all-DVE ---
        q = 0
        t_k0 = kpool.tile([P, WT], i32, tag="k")
        t_m0 = mpool.tile([P, WT], i32, tag="m")
        t_s0 = spool.tile([P, WT], bf16, tag="s")

        def t0_sub(sgi):
            nonlocal q
            gs, ge = 4 * sgi, 4 * sgi + 4
            for g in range(gs, ge):
                if sgi == 0:
                    eng = nc.vector
                else:
                    eng = nc.gpsimd if q % 2 == 0 else nc.vector
                    q += 1
                fma_item(t_k0, g, 0, eng)
            andor(t_m0, t_k0, int(off[gs]), int(off[ge]))
            sin(t_s0, t_m0, int(off[gs]), int(off[ge]))

        t_cw = {r: cwpool.tile([wc[r], 2 * N_COLS], f16, name=f"cb{r}")
                for r in classes}
        t_tmpo = opool.tile([P, N_COLS], f32)
        t_wav = opool.tile([P, N_COLS], f32)

        def stage2(lo, hi):  # chunks [lo, hi) are final
            for r in classes:
                nc.vector.tensor_copy(t_cw[r][:, 2 * lo:2 * hi],
                                      p_c[r][:, 2 * lo:2 * hi])
            if classes:
                for C in range(lo, hi):
                    for j, r in enumerate(classes):
                        nc.tensor.matmul(
                            p_fir[:RCH, C:C + 1], t_firm[r][:],
                            t_cw[r][:, 2 * C:2 * C + 1],
                            start=(j == 0), stop=False)
                        nc.tensor.matmul(
                            p_fir[:RCH, C:C + 1], t_firm2[r][:],
                            t_cw[r][:, 2 * C + 1:2 * C + 2],
                            start=False, stop=(j == len(classes) - 1))
            nc.vector.tensor_scalar(
                t_tmpo[:RCH, lo:hi], p_r[:RCH, 2 * lo + 1:2 * hi:2],
                t_w[:RCH, 0:1], None, mybir.AluOpType.mult)
            nc.vector.tensor_tensor(
                t_wav[:RCH, lo:hi], t_tmpo[:RCH, lo:hi],
                p_r[:RCH, 2 * lo:2 * hi:2], mybir.AluOpType.add)
            if classes:
                nc.vector.tensor_tensor(t_wav[:RCH, lo:hi],
                                        t_wav[:RCH, lo:hi],
                                        p_fir[:RCH, lo:hi],
                                        mybir.AluOpType.add)
            nc.sync.dma_start(d_out[:, lo:hi], t_wav[:RCH, lo:hi])

        def do_tile(t):
            nonlocal q
            t_k = kpool.tile([P, WT], i32, tag="k")
            for g in range(NB):
                if t <= 2:
                    eng = nc.gpsimd if q % 3 == 0 else nc.vector
                else:
                    eng = nc.gpsimd if (q * 3) % 5 < 3 else nc.vector
                q += 1
                fma_item(t_k, g, t, eng)
            t_m = mpool.tile([P, WT], i32, tag="m")
            t_s = spool.tile([P, WT], bf16, tag="s")
            andor(t_m, t_k, 0, WT)
            sin(t_s, t_m, 0, WT)
            reduce_tile(t_s, t)

        for sgi in range(4):
            t0_sub(sgi)
        reduce_tile(t_s0, 0)
        for t in range(1, N_TILES):
            do_tile(t)
            if t == 5:                   # chunks 0..24 are final
                stage2(0, 25)
            elif t == 8:                 # chunks 25..44 are final
                stage2(25, 45)
        stage2(45, N_COLS)

    nc.compile()
    return nc


def _prepare(amplitude_logit, tau, omega_logit, sigma_logit,
             phi_vector, gamma):
    al = amplitude_logit.astype(np.float64)
    tau = tau.astype(np.float64)
    ol = omega_logit.astype(np.float64)
    sl = sigma_logit.astype(np.float64)
    pv = phi_vector.astype(np.float64)
    gamma = gamma.astype(np.float64)

    amp = np.where(al > 30, al, np.log1p(np.exp(al)))
    omega = (1.0 / (1.0 + np.exp(-ol))) * 0.99 * NYQUIST
    sigma = np.where(sl > 30, sl, np.log1p(np.exp(sl))) + SIGMA_OFFSET
    phi = np.arctan2(pv[:, 1], pv[:, 0])

    order = np.argsort(omega, kind="stable")
    omega, tau, sigma, gamma, phi, amp = (
        x[order] for x in (omega, tau, sigma, gamma, phi, amp))

    rates = []
    for g in range(NB):
        om = omega[g * P:(g + 1) * P].max()
        r = 1
        for rr in (8, 4, 2):
            if om <= RATE_MARG * FS / rr:
                r = rr
                break
        rates.append(r)
    rates = tuple(rates)
    classes = sorted({r for r in rates if r > 1})
    wg = [TW // r + (0 if r == 1 else 2 * H) for r in rates]

    t_node = np.arange(N_COLS * N_CORES + 1) * RCH / FS      # global nodes

    in_maps = []
    for c in range(N_CORES):
        t0 = c * TC / FS
        bi = np.zeros((N_ATOMS, NB * N_TILES), np.float64)
        ci = np.zeros((N_ATOMS, NB * N_TILES), np.float64)
        # per block g, tile t: coarse window [t*TW/r - H*(r>1), ...]
        for g in range(NB):
            r = rates[g]
            sel = slice(g * P, (g + 1) * P)
            halo = 0 if r == 1 else H
            w = wg[g]
            for t in range(N_TILES):
                k0 = t * (TW // r) - halo          # first coarse idx
                kc = k0 + w // 2                   # ramp center
                t_c = t0 + kc * r / FS
                D = t_c - tau[sel]
                y = omega[sel] * D + gamma[sel] * D * D / (2 * np.pi) \
                    + phi[sel] / (2 * np.pi) + 0.25
                dy = omega[sel] + gamma[sel] * D / np.pi
                bi[sel, g * N_TILES + t] = np.round(dy * r / FS * FIX)
                ci[sel, g * N_TILES + t] = np.round(np.mod(y, 1.0) * FIX)

        nodes = t_node[c * N_COLS:(c + 1) * N_COLS + 1]
        E = amp[:, None] * np.exp(
            -0.5 * ((nodes[None, :] - tau[:, None]) / sigma[:, None]) ** 2)
        envp = np.empty((N_ATOMS, 2 * N_COLS), np.float64)
        envp[:, 0::2] = -E[:, :-1]
        envp[:, 1::2] = -(E[:, 1:] - E[:, :-1])
        # decimated blocks: env segment spans the halo'd coarse window
        wcd = {r: RCH // r + 2 * H for r in classes}
        for g in range(NB):
            r = rates[g]
            if r == 1:
                continue
            sel = slice(g * P, (g + 1) * P)
            Cs = np.arange(N_COLS)
            t_a = t0 + (Cs * RCH - H * r) / FS                  # col 0 time
            t_b = t_a + (wcd[r] - 1) * r / FS                   # last col
            Ea = amp[sel, None] * np.exp(
                -0.5 * ((t_a[None, :] - tau[sel, None]) / sigma[sel, None]) ** 2)
            Eb = amp[sel, None] * np.exp(
                -0.5 * ((t_b[None, :] - tau[sel, None]) / sigma[sel, None]) ** 2)
            envp[sel, 0::2] = -Ea
            envp[sel, 1::2] = -(Eb - Ea)

        def blk(x):  # [2048, W] -> [128, NB*W] block-major
            w = x.shape[1] // NB if x.shape[1] == NB * N_TILES else x.shape[1]
            out = np.empty((P, 0), x.dtype)
            cols = []
            for g in range(NB):
                cols.append(x[g * P:(g + 1) * P])
            return np.hstack(cols)

        # bi/ci already have per-block columns [g*N_TILES + t] but rows are
        # all 2048 atoms; select each block's rows for its columns
        bi_b = np.empty((P, NB * N_TILES), np.float32)
        ci_b = np.empty((P, NB * N_TILES), np.float32)
        for g in range(NB):
            bi_b[:, g * N_TILES:(g + 1) * N_TILES] = \
                bi[g * P:(g + 1) * P, g * N_TILES:(g + 1) * N_TILES]
            ci_b[:, g * N_TILES:(g + 1) * N_TILES] = \
                ci[g * P:(g + 1) * P, g * N_TILES:(g + 1) * N_TILES]
        env_b = np.empty((P, NB * 2 * N_COLS), bft)
        for g in range(NB):
            env_b[:, g * 2 * N_COLS:(g + 1) * 2 * N_COLS] = \
                envp[g * P:(g + 1) * P].astype(bft)

        m = {"bi": np.ascontiguousarray(bi_b),
             "ci": np.ascontiguousarray(ci_b),
             "envp": np.ascontiguousarray(env_b)}
        for r in classes:
            M = _fir_matrix(r)
            wcr = M.shape[0]
            Wck = (np.arange(wcr) / (wcr - 1.0))[:, None]
            m[f"firm{r}"] = np.ascontiguousarray(M.astype(np.float16))
            m[f"firm2{r}"] = np.ascontiguousarray((M * Wck).astype(np.float16))
        in_maps.append(m)
    return rates, in_maps


def kernel(amplitude_logit, tau, omega_logit, sigma_logit, phi_vector, gamma, t):
    rates, in_maps = _prepare(amplitude_logit, tau, omega_logit, sigma_logit,
                              phi_vector, gamma)
    if rates not in _cache:
        _cache[rates] = _build_program(rates)
    nc = _cache[rates]
    res = run_bass_kernel_spmd(nc, in_maps, list(range(N_CORES)))
    total = np.zeros(T, dtype=np.float64)
    for c, r in enumerate(res.results):
        wv = r["wave"].astype(np.float64)          # [RCH, N_COLS]
        total[c * TC:(c + 1) * TC] = wv.T.ravel()  # s = C*120 + i
    return total.astype(np.float32)



# revision 2
# speedup vs baseline: 5.7081x; 5.7081x over previous
"""Gabor-atom synthesis via exact Fourier-basis factorization, time-sharded
across 8 cores.

Each 120-sample chunk of the output is synthesized in a fixed 128-function
Fourier basis (cos/sin at bin spacing 2*pi/128 per sample). For each atom,
cos(beta*i) and sin(beta*i) over i in [0,120) are EXACTLY representable in
that basis (min-norm solve against B [120x128], cond(BB^T)~1.9); the chirp
(gamma) only drifts beta by ~1e-6 rad/sample over the whole signal, so one
per-atom coefficient vector works for every chunk with the per-chunk phase
alpha computed exactly on the host.

Per chunk c the device computes
    y_c[i] = sum_f B[i,f] * Z0[f,c]  +  (i/S) * sum_f B[i,f] * Zd[f,c]
    Z0[f,c] = sum_n P[n,f]*(e0*cos a)[n,c] + Qt[n,f]*(e0*sin a)[n,c]
(Zd likewise with the per-chunk envelope delta), i.e. two matmul stages:
16 atom-blocks x 2 stationary [128,128] contractions into PSUM bins, then a
single fixed [128x120] basis synthesis with the envelope ramp folded into a
second stationary matrix. No per-sample elementwise work on any engine.
"""
import numpy as np
from contextlib import ExitStack

import concourse.bacc as bacc
import concourse.tile as tile
from concourse import mybir
from concourse.bass_utils import run_bass_kernel_spmd

FS = 24000.0
T = 48000
N_ATOMS = 2048
N_CORES = 8
NYQUIST = FS / 2.0
SIGMA_OFFSET = 1e-3

P = 128                      # partitions / atoms per block
NB = 16                      # atom blocks
S = 120                      # samples per chunk
F = 128                      # basis functions
TC = T // N_CORES            # 6000 samples per core
NCH = TC // S                # 50 chunks per core
NPC = 4                      # DMA pieces (blocks NB/NPC per piece)
BPP = NB // NPC

f32 = mybir.dt.float32
f16 = mybir.dt.float16

_cache = {}


def _basis():
    i_ = np.arange(S)
    fc = np.arange(0, 65)
    fs_ = np.arange(1, 64)
    B = np.concatenate([np.cos(2 * np.pi * np.outer(i_, fc) / F),
                        np.sin(2 * np.pi * np.outer(i_, fs_) / F)], axis=1)
    M = B.T @ np.linalg.inv(B @ B.T)          # [F, S] min-norm projector
    return B, M


_B, _M = _basis()


def _build_program():
    nc = bacc.Bacc("TRN2", target_bir_lowering=False, debug=False)

    d_k = nc.dram_tensor("kmat", [P, NB * 2 * F], f16, kind="ExternalInput").ap()
    d_e = nc.dram_tensor("emat", [P, NB * 4 * NCH], f16, kind="ExternalInput").ap()
    d_b = nc.dram_tensor("bmat", [P, 2 * S], f16, kind="ExternalInput").ap()
    d_out = nc.dram_tensor("wave", [S, NCH], f32, kind="ExternalOutput").ap()

    KW = BPP * 2 * F          # k piece width (cols)
    EW = BPP * 4 * NCH        # e piece width

    with tile.TileContext(nc) as tc, ExitStack() as ctx:
        consts = ctx.enter_context(tc.tile_pool(name="consts", bufs=1))
        kpool = ctx.enter_context(tc.tile_pool(name="kp", bufs=1))
        epool = ctx.enter_context(tc.tile_pool(name="ep", bufs=1))
        zpool = ctx.enter_context(tc.tile_pool(name="zp", bufs=1))
        opool = ctx.enter_context(tc.tile_pool(name="op", bufs=1))
        rpool = ctx.enter_context(tc.tile_pool(name="rp", bufs=1, space="PSUM"))

        t_b = consts.tile([P, 2 * S], f16)
        nc.sync.dma_start(t_b[:], d_b[:])

        t_k = [kpool.tile([P, KW], f16, name=f"k{p}") for p in range(NPC)]
        t_e = [epool.tile([P, EW], f16, name=f"e{p}") for p in range(NPC)]
        for p in range(NPC):
            nc.sync.dma_start(t_k[p][:], d_k[:, p * KW:(p + 1) * KW])
            nc.sync.dma_start(t_e[p][:], d_e[:, p * EW:(p + 1) * EW])

        p_z = rpool.tile([P, 2 * NCH], f32)
        p_y = rpool.tile([P, NCH], f32)

        n_mm = 2 * NB
        n = 0
        for p in range(NPC):
            for g in range(BPP):
                ko = g * 2 * F
                eo = g * 4 * NCH
                nc.tensor.matmul(p_z[:, 0:2 * NCH],
                                 t_k[p][:, ko:ko + F],
                                 t_e[p][:, eo:eo + 2 * NCH],
                                 start=(n == 0), stop=False)
                n += 1
                nc.tensor.matmul(p_z[:, 0:2 * NCH],
                                 t_k[p][:, ko + F:ko + 2 * F],
                                 t_e[p][:, eo + 2 * NCH:eo + 4 * NCH],
                                 start=False, stop=(n == n_mm - 1))
                n += 1

        t_z = zpool.tile([P, 2 * NCH], f16)
        nc.vector.tensor_copy(t_z[:], p_z[:])

        nc.tensor.matmul(p_y[:S, :], t_b[:, 0:S], t_z[:, 0:NCH],
                         start=True, stop=False)
        nc.tensor.matmul(p_y[:S, :], t_b[:, S:2 * S], t_z[:, NCH:2 * NCH],
                         start=False, stop=True)

        t_y = opool.tile([P, NCH], f32)
        nc.vector.tensor_copy(t_y[:S, :], p_y[:S, :])
        nc.sync.dma_start(d_out[:, :], t_y[:S, :])

    nc.compile()
    return nc


def _prepare(amplitude_logit, tau, omega_logit, sigma_logit, phi_vector, gamma):
    al = amplitude_logit.astype(np.float64)
    tau = tau.astype(np.float64)
    ol = omega_logit.astype(np.float64)
    sl = sigma_logit.astype(np.float64)
    pv = phi_vector.astype(np.float64)
    gam = gamma.astype(np.float64)

    amp = np.where(al > 30, al, np.log1p(np.exp(al)))
    omega = (1.0 / (1.0 + np.exp(-ol))) * 0.99 * NYQUIST
    sigma = np.where(sl > 30, sl, np.log1p(np.exp(sl))) + SIGMA_OFFSET
    phi = np.arctan2(pv[:, 1], pv[:, 0])

    # per-atom frequency (rad/sample) at signal center; chirp drift over the
    # full 2 s is ~4e-6 rad/sample -> phase error < 5e-4 within any chunk
    beta = (2 * np.pi * omega + 2 * gam * (1.0 - tau)) / FS

    i_ = np.arange(S)
    Pc = np.cos(np.outer(beta, i_)) @ _M.T          # [N, F]
    Qt = -(np.sin(np.outer(beta, i_)) @ _M.T)       # [N, F] (sign folded)

    kmat = np.empty((P, NB * 2 * F), np.float16)
    for g in range(NB):
        sel = slice(g * P, (g + 1) * P)
        kmat[:, g * 2 * F:g * 2 * F + F] = Pc[sel].astype(np.float16)
        kmat[:, g * 2 * F + F:(g + 1) * 2 * F] = Qt[sel].astype(np.float16)

    bmat = np.empty((P, 2 * S), np.float16)
    bmat[:, 0:S] = _B.T.astype(np.float16)                      # B[f, i]
    bmat[:, S:2 * S] = (_B * (i_ / S)[:, None]).T.astype(np.float16)

    # envelope at all global chunk nodes and phase at all chunk starts
    nodes = np.arange(N_CORES * NCH + 1) * S / FS               # [401]
    G = amp[:, None] * np.exp(
        -0.5 * ((nodes[None, :] - tau[:, None]) / sigma[:, None]) ** 2)
    starts = np.arange(N_CORES * NCH) * S / FS                  # [400]
    dt = starts[None, :] - tau[:, None]
    ph = 2 * np.pi * omega[:, None] * dt + gam[:, None] * dt * dt + phi[:, None]
    ca = np.cos(ph)
    sa = np.sin(ph)
    e0 = G[:, :-1]
    de = G[:, 1:] - G[:, :-1]
    Ec0 = (e0 * ca).astype(np.float16)
    Ecd = (de * ca).astype(np.float16)
    Es0 = (e0 * sa).astype(np.float16)
    Esd = (de * sa).astype(np.float16)

    in_maps = []
    for c in range(N_CORES):
        cs = slice(c * NCH, (c + 1) * NCH)
        emat = np.empty((P, NB * 4 * NCH), np.float16)
        for g in range(NB):
            sel = slice(g * P, (g + 1) * P)
            o = g * 4 * NCH
            emat[:, o:o + NCH] = Ec0[sel, cs]
            emat[:, o + NCH:o + 2 * NCH] = Ecd[sel, cs]
            emat[:, o + 2 * NCH:o + 3 * NCH] = Es0[sel, cs]
            emat[:, o + 3 * NCH:o + 4 * NCH] = Esd[sel, cs]
        in_maps.append({"kmat": kmat, "emat": np.ascontiguousarray(emat),
                        "bmat": bmat})
    return in_maps


def kernel(amplitude_logit, tau, omega_logit, sigma_logit, phi_vector, gamma, t):
    in_maps = _prepare(amplitude_logit, tau, omega_logit, sigma_logit,
                       phi_vector, gamma)
    if "prog" not in _cache:
        _cache["prog"] = _build_program()
    nc = _cache["prog"]
    res = run_bass_kernel_spmd(nc, in_maps, list(range(N_CORES)))
    total = np.empty(T, dtype=np.float32)
    for c, r in enumerate(res.results):
        wv = r["wave"].astype(np.float32)          # [S, NCH]
        total[c * TC:(c + 1) * TC] = wv.T.ravel()  # s = C*120 + i
    return total
